# revision 1
# baseline (speedup 1.0000x reference)
"""GQA attention (B=2, S=2048, D=2048, Hq=32, Hkv=8, dh=64) on 8 TRN2 NeuronCores.

Sharding: tensor-parallel over head groups (4 shards: 8 q heads + 2 kv heads
each, GQA groups intact) x data-parallel over batch (2). Each core computes a
partial [S, D] output (its heads' contribution through its wo row-shard); the
host sums the 4 TP partials per batch. No on-device collectives.

Device layout is fully transposed ([dim, seq]): scores are computed as
S^T[k, q] so the PV matmul consumes exp(S^T) directly (no transposes), and the
softmax denominator is produced by a ones-column appended to V. Compute in
bf16 with f32 PSUM accumulation.
"""

import sys

for _p in ("/opt/trn_rl_repo",):
    if _p not in sys.path:
        sys.path.insert(0, _p)

import numpy as np
import ml_dtypes

import concourse.bass as bass
import concourse.tile as tile
from concourse import bacc, mybir
from concourse.bass_utils import run_bass_kernel_spmd

BF16 = ml_dtypes.bfloat16
FP32 = mybir.dt.float32
BF = mybir.dt.bfloat16

B, S, D = 2, 2048, 2048
NH, NKV, DH = 32, 8, 64
P = 128
TP = 4            # head-group shards
NQL = NH // TP    # 8 local q heads
NKVL = NKV // TP  # 2 local kv heads
OL = NQL * DH     # 512 local q-proj dims
KVL = NKVL * DH   # 128 local kv-proj dims
NDC = D // P      # 16 contraction chunks
NQJ = S // 512    # 4 q blocks of 512
NKI = S // P      # 16 k blocks of 128
SCALE = 1.0 / 8.0  # 1/sqrt(dh)
MASK_NEG = -1e8    # "fully masked" threshold


def _chunk_major(a, pchunks):
    """[pchunks*128, W] -> [128, pchunks*W] with chunk i at cols [i*W,(i+1)*W)."""
    n, w = a.shape
    assert n == pchunks * P
    return np.ascontiguousarray(
        a.reshape(pchunks, P, w).transpose(1, 0, 2).reshape(P, pchunks * w)
    )


def _rot_matrix():
    """R s.t. R @ v == rotate_half(v) for one 64-dim head; block-diag for 2."""
    R = np.zeros((DH, DH), np.float32)
    for i in range(DH // 2):
        R[i, i + DH // 2] = -1.0
    for i in range(DH // 2, DH):
        R[i, i - DH // 2] = 1.0
    R2 = np.zeros((P, P), np.float32)
    R2[:DH, :DH] = R
    R2[DH:, DH:] = R
    return R2


def _analyze_masks(causal_mask, padding_mask):
    """Classify 128(k) x 512(q) blocks of the transposed additive mask.

    Returns (plan, blobs): plan[qj] = list of (ki, a0, a1, moff) where
    (a0, a1) is the column interval needing a mask add (None if none) and
    moff the column offset into the per-core mask blob; blobs[b] is the
    [128, W] f32 tile data (already scaled by 8) for batch b.
    """
    cmT = np.stack(
        [
            (causal_mask[0, 0] + padding_mask[b, 0, 0][None, :]).T.astype(np.float32)
            for b in range(B)
        ]
    )  # [B, k, q]
    plan = []
    tiles = [[] for _ in range(B)]
    moff = 0
    for qj in range(NQJ):
        row = []
        for ki in range(NKI):
            blk = cmT[:, ki * P : (ki + 1) * P, qj * 512 : (qj + 1) * 512]
            if np.all(blk <= MASK_NEG):
                continue  # contributes exactly zero probability everywhere
            if np.all(blk == 0.0):
                row.append((ki, 0, None, None, None))
                continue
            dead = np.all(blk <= MASK_NEG, axis=(0, 1))  # per-column fully masked
            l0 = 0
            while l0 < 512 and dead[l0]:
                l0 += 1
            colneed = np.any(blk != 0.0, axis=(0, 1))
            colneed[:l0] = False  # dead prefix is skipped, not masked
            idx = np.nonzero(colneed)[0]
            if len(idx) == 0:
                row.append((ki, l0, None, None, None))
                continue
            a0, a1 = int(idx[0]), int(idx[-1]) + 1
            for b in range(B):
                tiles[b].append(
                    np.ascontiguousarray(8.0 * blk[b, :, a0:a1], np.float32)
                )
            row.append((ki, l0, a0, a1, moff))
            moff += a1 - a0
        plan.append(row)
    if moff == 0:
        blobs = [np.zeros((P, 1), np.float32) for _ in range(B)]
    else:
        blobs = [np.concatenate(t, axis=1) for t in tiles]
    return plan, blobs


def _build(plan, mask_w, use_bias, dbg=False, stages="all"):
    """Build the single-core SPMD graph. Returns the compiled Bacc."""
    nc = bacc.Bacc("TRN2", target_bir_lowering=False, debug=False, num_devices=8)

    xT = nc.dram_tensor("xT", [P, NDC * S], BF, kind="ExternalInput").ap()
    wqT = nc.dram_tensor("wqT", [P, NDC * OL], BF, kind="ExternalInput").ap()
    wkT = nc.dram_tensor("wkT", [P, NDC * KVL], BF, kind="ExternalInput").ap()
    wvT = nc.dram_tensor("wvT", [P, NDC * KVL], BF, kind="ExternalInput").ap()
    woT = nc.dram_tensor("woT", [P, 4 * D], BF, kind="ExternalInput").ap()
    cosT = nc.dram_tensor("cosT", [P, S], BF, kind="ExternalInput").ap()
    sinT = nc.dram_tensor("sinT", [P, S], BF, kind="ExternalInput").ap()
    r2T = nc.dram_tensor("r2T", [P, P], BF, kind="ExternalInput").ap()
    masks = nc.dram_tensor("masks", [P, mask_w], FP32, kind="ExternalInput").ap()
    if use_bias:
        # pre-broadcast on host: row-replicated bias tiles, DMA'd directly
        biasT = nc.dram_tensor("biasT", [P, NQL * S], FP32, kind="ExternalInput").ap()
    out = nc.dram_tensor("out", [S, D], FP32, kind="ExternalOutput").ap()
    if dbg:
        dbg_q = nc.dram_tensor("dbg_q", [P, S], BF, kind="ExternalOutput").ap()
        dbg_k = nc.dram_tensor("dbg_k", [P, S], BF, kind="ExternalOutput").ap()
        dbg_v = nc.dram_tensor("dbg_v", [P, 2 * (DH + 1)], BF, kind="ExternalOutput").ap()
        dbg_a = nc.dram_tensor("dbg_a", [P, S], BF, kind="ExternalOutput").ap()
        dbg_bc = nc.dram_tensor("dbg_bc", [64, 512], FP32, kind="ExternalOutput").ap()

    with tile.TileContext(nc) as tc:
        with (
            tc.tile_pool(name="const", bufs=1) as cpool,
            tc.tile_pool(name="acts", bufs=1) as apool,
        ):
            # Resident inputs. Chunked DMAs so the first projection matmuls
            # can start as soon as their chunk lands (not after 8MB).
            wkTs = cpool.tile([P, NDC * KVL], BF, tag="wkT")
            nc.sync.dma_start(wkTs[:], wkT[:])
            xTs = cpool.tile([P, NDC * S], BF, tag="xT")
            for d in range(NDC):
                nc.sync.dma_start(
                    xTs[:, d * S : (d + 1) * S], xT[:, d * S : (d + 1) * S]
                )
            cosTs = cpool.tile([P, S], BF, tag="cosT")
            nc.sync.dma_start(cosTs[:], cosT[:])
            sinTs = cpool.tile([P, S], BF, tag="sinT")
            nc.sync.dma_start(sinTs[:], sinT[:])
            r2Ts = cpool.tile([P, P], BF, tag="r2T")
            nc.sync.dma_start(r2Ts[:], r2T[:])
            wvTs = cpool.tile([P, NDC * KVL], BF, tag="wvT")
            nc.sync.dma_start(wvTs[:], wvT[:])
            wqTs = cpool.tile([P, NDC * OL], BF, tag="wqT")
            for d in range(4):
                nc.sync.dma_start(
                    wqTs[:, d * 4 * OL : (d + 1) * 4 * OL],
                    wqT[:, d * 4 * OL : (d + 1) * 4 * OL],
                )
            woTs = cpool.tile([P, 4 * D], BF, tag="woT")
            nc.sync.dma_start(woTs[:], woT[:])
            ones1 = cpool.tile([1, 64], BF, tag="ones1")
            nc.vector.memset(ones1[:], 1.0)

            # Products of the projection phase
            qropeT = [apool.tile([P, S], BF, tag=f"qrope{oc}", name=f"qrope{oc}") for oc in range(4)]
            kropeT = apool.tile([P, S], BF, tag="krope", name="krope")
            vaug = [apool.tile([P, 2 * (DH + 1)], BF, tag=f"v{si}", name=f"v{si}") for si in range(NKI)]
            attnT = [apool.tile([P, S], BF, tag=f"attnT{oc}", name=f"attnT{oc}") for oc in range(4)]

            # ---------------- Phase 1: projections + rope ----------------
            with (
                tc.tile_pool(name="pp", bufs=5, space="PSUM") as pp,
                tc.tile_pool(name="pr", bufs=3, space="PSUM") as pr,
                tc.tile_pool(name="ropesb", bufs=5) as rsb,
                tc.tile_pool(name="ropetmp", bufs=6) as rtmp,
            ):
                def project_rope(dst, w_sb, w_cols, oc, sj):
                    """dst[:, sj*512:(sj+1)*512] = rope of (w.T @ x.T) slice."""
                    ps = pp.tile([P, 512], FP32, tag="proj", name="psp")
                    for d in range(NDC):
                        nc.tensor.matmul(
                            ps[:],
                            w_sb[:, d * w_cols + oc * P : d * w_cols + (oc + 1) * P],
                            xTs[:, d * S + sj * 512 : d * S + (sj + 1) * 512],
                            start=(d == 0),
                            stop=(d == NDC - 1),
                        )
                    qb = rsb.tile([P, 512], BF, tag="qb", name="qb")
                    nc.scalar.copy(qb[:], ps[:])
                    pz = pr.tile([P, 512], FP32, tag="rot", name="psz")
                    nc.tensor.matmul(pz[:], r2Ts[:], qb[:], start=True, stop=True)
                    t1 = rtmp.tile([P, 512], FP32, tag="t1", name="t1")
                    nc.vector.tensor_tensor(
                        t1[:], ps[:], cosTs[:, sj * 512 : (sj + 1) * 512],
                        op=mybir.AluOpType.mult,
                    )
                    t2 = rtmp.tile([P, 512], FP32, tag="t2", name="t2")
                    nc.vector.tensor_tensor(
                        t2[:], pz[:], sinTs[:, sj * 512 : (sj + 1) * 512],
                        op=mybir.AluOpType.mult,
                    )
                    nc.vector.tensor_tensor(
                        dst[:, sj * 512 : (sj + 1) * 512], t1[:], t2[:],
                        op=mybir.AluOpType.add,
                    )

                def emit_v(si):
                    pv = pp.tile([P, P], FP32, tag="proj", name="psv")
                    for d in range(NDC):
                        nc.tensor.matmul(
                            pv[:],
                            xTs[:, d * S + si * P : d * S + (si + 1) * P],
                            wvTs[:, d * KVL : (d + 1) * KVL],
                            start=(d == 0),
                            stop=(d == NDC - 1),
                        )
                    vt = vaug[si]
                    nc.scalar.copy(vt[:, 0:DH], pv[:, 0:DH])
                    nc.scalar.copy(vt[:, DH + 1 : 2 * DH + 1], pv[:, DH : 2 * DH])
                    nc.vector.memset(vt[:, DH : DH + 1], 1.0)
                    nc.vector.memset(vt[:, 2 * DH + 1 : 2 * DH + 2], 1.0)

                for sj in range(4):
                    project_rope(kropeT, wkTs, KVL, 0, sj)
                for si in range(4):
                    emit_v(si)
                for sj in range(2):
                    for oc in range(4):
                        project_rope(qropeT[oc], wqTs, OL, oc, sj)

            # ---------------- Phase 2: attention + wo ----------------
            with (
                tc.tile_pool(name="ps", bufs=4, space="PSUM") as psc,
                tc.tile_pool(name="pa", bufs=2, space="PSUM") as pat,
                tc.tile_pool(name="pw", bufs=2, space="PSUM") as pwo,
                tc.tile_pool(name="probs", bufs=(6 if use_bias else 8)) as prb,
                tc.tile_pool(name="maskt", bufs=(3 if use_bias else 4)) as mpool,
                tc.tile_pool(name="norm", bufs=3) as npool,
                tc.tile_pool(name="osb", bufs=(4 if use_bias else 6)) as opool,
                tc.tile_pool(name="np2q", bufs=1) as np2q,
            ):
                def qproj2_gen(dst, w_sb, w_cols, oc, sj):
                    # deferred projection inside phase 2: proj psum borrows the
                    # wo pool (idle until qj=1), rot borrows a scores slot
                    ps = pwo.tile([P, 512], FP32, tag="wo", name="psp2")
                    for d in range(NDC):
                        nc.tensor.matmul(
                            ps[:],
                            w_sb[:, d * w_cols + oc * P : d * w_cols + (oc + 1) * P],
                            xTs[:, d * S + sj * 512 : d * S + (sj + 1) * 512],
                            start=(d == 0),
                            stop=(d == NDC - 1),
                        )
                        if d % 2 == 1:
                            yield
                    qb = np2q.tile([P, 512], BF, tag="qb2", name="qb2", bufs=2)
                    nc.vector.tensor_copy(qb[:], ps[:])
                    pz = psc.tile([P, 512], FP32, tag="sT", name="psz2")
                    nc.tensor.matmul(pz[:], r2Ts[:], qb[:], start=True, stop=True)
                    yield
                    t1 = np2q.tile([P, 512], FP32, tag="t1b", name="t1b", bufs=2)
                    nc.vector.tensor_tensor(
                        t1[:], ps[:], cosTs[:, sj * 512 : (sj + 1) * 512],
                        op=mybir.AluOpType.mult,
                    )
                    t2 = np2q.tile([P, 512], FP32, tag="t2b", name="t2b", bufs=1)
                    nc.vector.tensor_tensor(
                        t2[:], pz[:], sinTs[:, sj * 512 : (sj + 1) * 512],
                        op=mybir.AluOpType.mult,
                    )
                    nc.vector.tensor_tensor(
                        dst[:, sj * 512 : (sj + 1) * 512], t1[:], t2[:],
                        op=mybir.AluOpType.add,
                    )
                    yield

                def v2_gen(si):
                    pv = pwo.tile([P, P], FP32, tag="wo", name="psv2")
                    for d in range(NDC):
                        nc.tensor.matmul(
                            pv[:],
                            xTs[:, d * S + si * P : d * S + (si + 1) * P],
                            wvTs[:, d * KVL : (d + 1) * KVL],
                            start=(d == 0),
                            stop=(d == NDC - 1),
                        )
                        if d % 2 == 1:
                            yield
                    vt = vaug[si]
                    nc.vector.tensor_copy(vt[:, 0:DH], pv[:, 0:DH])
                    nc.vector.tensor_copy(vt[:, DH + 1 : 2 * DH + 1], pv[:, DH : 2 * DH])
                    nc.vector.memset(vt[:, DH : DH + 1], 1.0)
                    nc.vector.memset(vt[:, 2 * DH + 1 : 2 * DH + 2], 1.0)
                    yield

                def wo_gen(si, oc_order=(0, 1, 2, 3), djs=(0, 1, 2, 3)):
                    for dj in djs:
                        po = pwo.tile([P, 512], FP32, tag="wo", name="po")
                        for j, oc in enumerate(oc_order):
                            nc.tensor.matmul(
                                po[:],
                                attnT[oc][:, si * P : (si + 1) * P],
                                woTs[:, oc * D + dj * 512 : oc * D + (dj + 1) * 512],
                                start=(j == 0),
                                stop=(j == 3),
                            )
                            yield
                        ot = opool.tile([P, 512], FP32, tag="ot", name="ot")
                        nc.vector.tensor_copy(ot[:], po[:])
                        nc.sync.dma_start(
                            out[si * P : (si + 1) * P, dj * 512 : (dj + 1) * 512],
                            ot[:],
                        )
                        yield

                def emit_wo(si, oc_order=(0, 1, 2, 3)):
                    for _ in wo_gen(si, oc_order):
                        pass

                def wo_half_gen(qj, g, oc_order=(0, 1, 2, 3)):
                    si = (qj - 1) * 4 + g // 2
                    djs = (0, 1) if g % 2 == 0 else (2, 3)
                    return wo_gen(si, oc_order, djs)

                def head_scores(qj, g, fgen=None):
                    # head packing: qrope/attnT tile oc holds local heads
                    # (oc, oc+4) at partition halves (0, 1); a head's half
                    # equals its local kv index, aligning matmul operands.
                    def pump():
                        if fgen is not None:
                            next(fgen, None)

                    kis = plan[qj]
                    kv = g // 4
                    oc = g % 4
                    prow = 64 * (g // 4)
                    pacc = pat.tile([P, 512], FP32, tag="attn", name="pacc")
                    pts = []
                    for idx, (ki, l0, a0, a1, moff) in enumerate(kis):
                        ps = psc.tile([P, 512], FP32, tag="sT", name="pss")
                        nc.tensor.matmul(
                            ps[:, l0:512],
                            kropeT[64 * kv : 64 * kv + 64, ki * P : (ki + 1) * P],
                            qropeT[oc][prow : prow + 64,
                                       qj * 512 + l0 : (qj + 1) * 512],
                            start=True,
                            stop=True,
                        )
                        if use_bias:
                            bb = npool.tile([P, 512], FP32, tag="bb", name="bb", bufs=3)
                            nc.sync.dma_start(
                                bb[:, l0:512],
                                biasT[:, g * S + qj * 512 + l0
                                      : g * S + (qj + 1) * 512],
                            )
                            nc.vector.tensor_tensor(
                                ps[:, l0:512], ps[:, l0:512], bb[:, l0:512],
                                op=mybir.AluOpType.subtract,
                            )
                        if a0 is not None:
                            mt = mpool.tile([P, 512], FP32, tag="mask", name="mt")
                            nc.sync.dma_start(
                                mt[:, 0 : a1 - a0],
                                masks[:, moff : moff + (a1 - a0)],
                            )
                            nc.vector.tensor_tensor(
                                ps[:, a0:a1], ps[:, a0:a1], mt[:, 0 : a1 - a0],
                                op=mybir.AluOpType.add,
                            )
                        pt = prb.tile([P, 512], BF, tag="pT", name="pt")
                        nc.scalar.activation(
                            pt[:, l0:512], ps[:, l0:512],
                            mybir.ActivationFunctionType.Exp,
                            scale=SCALE,
                        )
                        pts.append((pt, ki, l0))
                        pump()
                        pump()
                    if fgen is not None:
                        for _ in fgen:
                            pass
                    return (pts, pacc, kv, oc, prow, qj, g)

                def head_pv_gen(st):
                    pts, pacc, kv, oc, prow, qj, g = st
                    for idx, (pt, ki, l0) in enumerate(pts):
                        nc.tensor.matmul(
                            pacc[0 : DH + 1, l0:512],
                            vaug[ki][:, (DH + 1) * kv : (DH + 1) * (kv + 1)],
                            pt[:, l0:512],
                            start=(idx == 0),
                            stop=(idx == len(pts) - 1),
                        )
                        yield
                    # free the attn psum slot fast: one copy out, then the
                    # normalize chain runs entirely from SBUF
                    stg = npool.tile([DH + 1, 512], FP32, tag="stg", name="stg")
                    nc.vector.tensor_copy(stg[:], pacc[0 : DH + 1, :])
                    yield
                    rc = npool.tile([1, 512], FP32, tag="rc", name="rc")
                    nc.vector.reciprocal(rc[:], stg[DH : DH + 1, :])
                    bc = npool.tile([64, 512], FP32, tag="bc", name="bc")
                    nc.gpsimd.partition_broadcast(bc[:], rc[0:1, :])
                    if dbg and qj == 0 and g == 0:
                        nc.sync.dma_start(dbg_bc[:], bc[:])
                    nc.vector.tensor_tensor(
                        attnT[oc][prow : prow + 64, qj * 512 : (qj + 1) * 512],
                        stg[0:DH, :],
                        bc[:],
                        op=mybir.AluOpType.mult,
                    )
                    yield

                pending = None
                for qj in range(NQJ) if stages != "proj" else []:
                    # last block: order heads so attnT tiles finish in the
                    # order the trailing wo chains consume them
                    gorder = [3, 7, 2, 6, 1, 5, 0, 4] if qj == NQJ - 1 else range(NQL)
                    for pos, g in enumerate(gorder):
                        gens = []
                        if pending is not None:
                            gens.append(pending)
                        if qj == 0:
                            gens.append(v2_gen(4 + g))
                            gens.append(
                                qproj2_gen(qropeT[g % 4], wqTs, OL, g % 4,
                                           2 + g // 4)
                            )
                        elif qj == 1 and g >= 6:
                            gens.append(v2_gen(6 + g))
                        elif qj == 2 and g < 2:
                            gens.append(v2_gen(14 + g))
                        if stages == "all" and qj > 0:
                            gens.append(wo_half_gen(qj, g))
                        def chain(gs=gens):
                            for g_ in gs:
                                yield from g_
                        st = head_scores(qj, g, chain() if gens else None)
                        pending = head_pv_gen(st)
                if pending is not None:
                    for _ in pending:
                        pass
                if stages == "all":
                    for si in range((NQJ - 1) * 4, NQJ * 4):
                        emit_wo(si, oc_order=(3, 2, 1, 0))
                if dbg:
                    nc.sync.dma_start(dbg_q[:], qropeT[0][:])
                    nc.sync.dma_start(dbg_k[:], kropeT[:])
                    nc.sync.dma_start(dbg_v[:], vaug[0][:])
                    nc.sync.dma_start(dbg_a[:], attnT[0][:])

    nc.compile()
    return nc


def _host_prep(inputs):
    """Shard + transpose + cast everything; returns (in_maps, plan, use_bias)."""
    x = np.asarray(inputs["x"], np.float32)
    cos = np.asarray(inputs["cos"], np.float32)
    sin = np.asarray(inputs["sin"], np.float32)
    causal = np.asarray(inputs["causal_mask"], np.float32)
    padding = np.asarray(inputs["padding_mask"], np.float32)
    wq = np.asarray(inputs["wq"], np.float32)
    wk = np.asarray(inputs["wk"], np.float32)
    wv = np.asarray(inputs["wv"], np.float32)
    wo = np.asarray(inputs["wo"], np.float32)

    plan, mask_blobs = _analyze_masks(causal, padding)

    cosT = np.ascontiguousarray(np.concatenate([cos.T, cos.T], axis=0)).astype(BF16)
    sinT = np.ascontiguousarray(np.concatenate([sin.T, sin.T], axis=0)).astype(BF16)
    r2T = _rot_matrix().T.astype(BF16)  # lhsT for rot = R2 @ v

    # Overflow-safety check: exp without max-subtraction needs max score < ~60.
    # Cheap Cauchy-Schwarz bound first; exact (host) row max only if needed.
    use_bias = False
    bias_per_core = None
    qf = x.reshape(B * S, D) @ wq.T  # [B*S, NH*DH]
    kf = x.reshape(B * S, D) @ wk.T
    q4 = qf.reshape(B, S, NH, DH)
    k4 = kf.reshape(B, S, NKV, DH)
    c = cos[None, :, None, :]
    s_ = sin[None, :, None, :]

    def rot_half(t):
        t1, t2 = np.split(t, 2, axis=-1)
        return np.concatenate([-t2, t1], axis=-1)

    q4 = q4 * c + rot_half(q4) * s_
    k4 = k4 * c + rot_half(k4) * s_
    qn = np.linalg.norm(q4, axis=-1).max()
    kn = np.linalg.norm(k4, axis=-1).max()
    bound = qn * kn * SCALE
    if bound >= 60.0:
        # Exact per-(b, head, q) post-mask row max (like jax softmax), so the
        # largest unmasked exponent is exactly 0.
        mx = np.zeros((B, NH, S), np.float32)
        for b in range(B):
            cmask = (causal[0, 0] + padding[b, 0, 0][None, :]).astype(np.float32)
            for h in range(NH):
                sc = (q4[b, :, h, :] @ k4[b, :, h // 4, :].T) * SCALE + cmask
                mx[b, h] = sc.max(axis=1)
        bias_per_core = 8.0 * mx  # raw (pre-scale) units
        use_bias = True

    # local head permutation: tile oc holds heads (oc, oc+4) -> col order
    HPERM = [0, 4, 1, 5, 2, 6, 3, 7]
    in_maps = []
    for c_id in range(8):
        tp, b = c_id % TP, c_id // TP
        xT = _chunk_major(np.ascontiguousarray(x[b].T), NDC).astype(BF16)
        wq_l = wq.T[:, OL * tp : OL * (tp + 1)].reshape(D, NQL, DH)[:, HPERM, :]
        wqTs = _chunk_major(
            np.ascontiguousarray(wq_l.reshape(D, OL)), NDC
        ).astype(BF16)
        wkTs = _chunk_major(
            np.ascontiguousarray(wk.T[:, KVL * tp : KVL * (tp + 1)]), NDC
        ).astype(BF16)
        wvTs = _chunk_major(
            np.ascontiguousarray(wv.T[:, KVL * tp : KVL * (tp + 1)]), NDC
        ).astype(BF16)
        wo_l = wo.T[OL * tp : OL * (tp + 1), :].reshape(NQL, DH, D)[HPERM]
        woTs = _chunk_major(
            np.ascontiguousarray(wo_l.reshape(OL, D)), 4
        ).astype(BF16)
        m = {
            "xT": xT,
            "wqT": wqTs,
            "wkT": wkTs,
            "wvT": wvTs,
            "woT": woTs,
            "cosT": cosT,
            "sinT": sinT,
            "r2T": np.ascontiguousarray(r2T),
            "masks": mask_blobs[b],
        }
        if use_bias:
            bl = bias_per_core[b, NQL * tp : NQL * (tp + 1), :].reshape(1, NQL * S)
            m["biasT"] = np.ascontiguousarray(
                np.broadcast_to(bl, (P, NQL * S)), dtype=np.float32
            )
        in_maps.append(m)
    return in_maps, plan, mask_blobs[0].shape[1], use_bias


_CACHE = {}


def _get_compiled(plan_key, plan, mask_w, use_bias):
    if plan_key not in _CACHE:
        _CACHE[plan_key] = _build(plan, mask_w, use_bias)
    return _CACHE[plan_key]


def kernel(**inputs) -> np.ndarray:
    in_maps, plan, mask_w, use_bias = _host_prep(inputs)
    plan_key = (str(plan), mask_w, use_bias)
    nc = _get_compiled(plan_key, plan, mask_w, use_bias)
    res = run_bass_kernel_spmd(nc, in_maps, core_ids=list(range(8))).results
    out = np.zeros((B, S, D), np.float32)
    for c_id in range(8):
        tp, b = c_id % TP, c_id // TP
        out[b] += res[c_id]["out"]
    return out

